# revision 13
# baseline (speedup 1.0000x reference)
"""Trainium2 Bass kernel for the patch-retrieval module (nn_DMB_46737834115118).

Sharding: 8 cores = 4 batch samples x 2 image-row halves (hp/sim/argmax for
the local 512 patches; conv/grad/tables over the full sample).

All device work happens in PATCH layout; the host does the (pure
permutation) image<->patch transforms:
  - hst:  hs rearranged so each (n, half) gives a [128ch x 128pix] stationary
          block; the projection matmul (moving = spectral matrix, 3 cols)
          emits hs_s directly TRANSPOSED into hpT chunks (full fp32).
  - msh:  18x18 halo patches of ms; conv + sobel gradient run on DVE/ACT/Pool
          as per-partition shift-FMAs (no image-layout shuffles).
  - msp:  patchified ms (gather table).
Outputs are written in patch layout ([512, 768] sections) and the host
unpatchifies. The sim/argmax path is exact fp32 (argmax ties are fatal);
the soft matmul runs in fp32r and the gather tables (output-only data) in
bf16 — both far inside the 2e-2 tolerance.

DMA queues: hs stream exclusively on sync/SP; tables+outputs on scalar/ACT;
gathers on gpsimd/Pool - so nothing head-of-line blocks the hs stream.
"""
import numpy as np

import concourse.bass as bass
import concourse.tile as tile
from concourse import bacc, mybir
from concourse.bass_utils import run_bass_kernel_spmd
from concourse.masks import make_identity

F32 = mybir.dt.float32
F32R = mybir.dt.float32r
BF16 = mybir.dt.bfloat16
U32 = mybir.dt.uint32
AX = mybir.AxisListType
OP = mybir.AluOpType
ACTF = mybir.ActivationFunctionType

P = 128
NLOC = 512       # local patches per core
NFULL = 1024     # full-sample patches
D = 768          # 3*16*16
NT = 16          # n's per hs tile
NTILES = NLOC // NT          # 32
TPR = NTILES // 4            # tiles per round (8)


def _ap(base, off, dims):
    return bass.AP(base.tensor, base.offset + off, [list(d) for d in dims])


def _build():
    nc = bacc.Bacc("TRN2", target_bir_lowering=False, debug=False,
                   num_devices=8)

    hst_d = nc.dram_tensor("hst", [P, NLOC * 256], F32,
                           kind="ExternalInput").ap()
    msh_d = nc.dram_tensor("msh", [P, 8 * 972], F32,
                           kind="ExternalInput").ap()
    msp_d = nc.dram_tensor("msp", [NFULL, D], F32, kind="ExternalInput").ap()
    spec_d = nc.dram_tensor("spec", [P, 3], F32, kind="ExternalInput").ap()
    wcv_d = nc.dram_tensor("wcv", [P, 27], F32, kind="ExternalInput").ap()

    gradt = nc.dram_tensor("gradt", [NFULL, D], BF16, kind="Internal").ap()
    msdt = nc.dram_tensor("msdt", [NFULL, D], BF16, kind="Internal").ap()

    osoft_d = nc.dram_tensor("osoft", [NLOC, D], F32,
                             kind="ExternalOutput").ap()
    oms_d = nc.dram_tensor("oms", [NLOC, D], F32, kind="ExternalOutput").ap()
    ogr_d = nc.dram_tensor("ogr", [NLOC, D], F32, kind="ExternalOutput").ap()
    odf_d = nc.dram_tensor("odf", [NLOC, D], F32, kind="ExternalOutput").ap()

    with tile.TileContext(nc) as tc:
        with tc.tile_pool(name="const", bufs=1) as constp, \
             tc.tile_pool(name="persist", bufs=1) as pers, \
             tc.tile_pool(name="hstream", bufs=3) as hstr, \
             tc.tile_pool(name="stream", bufs=2) as strm, \
             tc.tile_pool(name="gbuf", bufs=2) as gbuf, \
             tc.tile_pool(name="pproj", bufs=1, space="PSUM") as psp_pr, \
             tc.tile_pool(name="pmain", bufs=2, space="PSUM") as psp_mm, \
             tc.tile_pool(name="ptr", bufs=2, space="PSUM") as psp_tr:

            spec_s = constp.tile([P, 3], F32, tag="spec")
            wcv_s = constp.tile([P, 27], F32, tag="wcv")
            ident = constp.tile([P, P], F32, tag="ident")
            eps_s = constp.tile([P, 1], F32, tag="eps")

            msh = pers.tile([P, 8 * 972], F32, tag="msh")
            mp = pers.tile([P, 8 * D], F32, tag="mp")          # [p,(mc,d)]
            hpT = [pers.tile([P, NLOC], F32, tag=f"hpT{k}", name=f"hpT{k}")
                   for k in range(6)]
            mpT = [pers.tile([P, NFULL], F32, tag=f"mpT{k}", name=f"mpT{k}")
                   for k in range(6)]
            PT = [pers.tile([P, P], F32R, tag=f"PT{k}", name=f"PT{k}")
                  for k in range(8)]
            gv = pers.tile([P, 2048], F32, tag="gv")
            gh = pers.tile([P, 2048], F32, tag="gh")
            gvb = pers.tile([P, 3 * 2048], BF16, tag="gvb")
            gate = pers.tile([P, 1], F32, tag="gate")
            recip_t = [pers.tile([P, 1], F32, tag=f"rc{k}", name=f"rc{k}")
                       for k in range(4)]
            idx_t = [pers.tile([P, 8], U32, tag=f"ix{k}", name=f"ix{k}")
                     for k in range(4)]
            ppr = [psp_pr.tile([P, 384], F32, tag=f"ppr{h}", name=f"ppr{h}")
                   for h in range(2)]

            # ---- initial DMAs: first hs tiles before everything else ----
            def load_hs(t):
                hs_sb = hstr.tile([P, NT * 256], F32, tag="hs")
                nc.sync.dma_start(hs_sb[:],
                                  hst_d[:, NT * 256 * t: NT * 256 * (t + 1)])
                return hs_sb

            tiles = {0: load_hs(0)}
            nc.scalar.dma_start(spec_s[:], spec_d[:])
            nc.scalar.dma_start(wcv_s[:], wcv_d[:])
            nc.scalar.dma_start(msh[:], msh_d[:])
            nc.vector.memset(eps_s[:], 1e-6)
            make_identity(nc, ident[:])

            # ---------------- conv (patch space, DVE only) ----------------
            def halo_in(c, di, dj):
                return _ap(msh[:], 324 * c + 18 * di + dj,
                           [list(msh[:].ap[0]), [972, 8], [18, 16], [1, 16]])

            def emit_conv():
                for c in range(3):
                    out = _ap(mp[:], 256 * c,
                              [list(mp[:].ap[0]), [D, 8], [1, 256]])
                    first = True
                    for di in range(3):
                        for dj in range(3):
                            wap = wcv_s[:, 9 * c + 3 * di + dj:
                                        9 * c + 3 * di + dj + 1]
                            src = halo_in(c, di, dj)
                            if first:
                                nc.vector.tensor_scalar_mul(out, src, wap)
                                first = False
                            else:
                                nc.vector.scalar_tensor_tensor(
                                    out, src, wap, out,
                                    op0=OP.mult, op1=OP.add)

            # grad + tables (emitted late so ACT stays free early)
            def emit_grad_tables():
                for c in range(3):
                    nc.vector.tensor_tensor(gv[:], halo_in(c, 2, 1),
                                            halo_in(c, 0, 1), op=OP.subtract)
                    nc.vector.tensor_tensor(gh[:], halo_in(c, 1, 2),
                                            halo_in(c, 1, 0), op=OP.subtract)
                    nc.scalar.square(gv[:], gv[:])
                    nc.vector.tensor_tensor(gh[:], gh[:], gh[:], op=OP.mult)
                    nc.vector.tensor_tensor(gv[:], gv[:], gh[:], op=OP.add)
                    nc.scalar.activation(gvb[:, 2048 * c: 2048 * (c + 1)],
                                         gv[:], ACTF.Sqrt,
                                         bias=eps_s[:, 0:1])

            def emit_tables():
                # gate Pool's deferred DMAs behind the round-3 evacuation
                # (real dep on the last hs tile) so they fill the tail window
                nc.gpsimd.tensor_copy(gate[:], hpT[5][:, 511:512])
                for c in range(3):
                    dst = _ap(gradt[:], 256 * c,
                              [[D, P], [P * D, 8], [1, 256]])
                    src = _ap(gvb[:], 2048 * c,
                              [list(gvb[:].ap[0]), [256, 8], [1, 256]])
                    nc.scalar.dma_start(dst, src)
                for g in range(4):          # msdt in 4 pieces (2 chunks
                    # each), f32 -> bf16 cast during SWDGE DMA
                    dstm = _ap(msdt[:], 2 * P * D * g,
                               [[D, P], [P * D, 2], [1, D]])
                    srcm = _ap(mp[:], 2 * D * g,
                               [list(mp[:].ap[0]), [D, 2], [1, D]])
                    nc.gpsimd.dma_start(dstm, srcm)

            def emit_mpT():
                for kc in range(6):
                    for j in range(8):
                        tp = psp_tr.tile([P, P], F32, tag="tp")
                        nc.tensor.transpose(
                            tp[:], mp[:, D * j + P * kc: D * j + P * kc + P],
                            ident[:])
                        nc.vector.tensor_copy(
                            mpT[kc][:, P * j: P * j + P], tp[:])
                # all exact consumers of mp are emitted; round in place for
                # the (tolerance-loose) fp32r soft matmuls
                nc.vector.tensor_copy(mp[:].bitcast(F32R), mp[:])

            # ---------------- hs projection for one tile ----------------
            def emit_proj(t):
                hs_sb = tiles.pop(t)
                if t + 1 < NTILES:
                    tiles[t + 1] = load_hs(t + 1)
                for i in range(NT):
                    n = NT * t + i
                    for h in range(2):
                        outap = _ap(ppr[h][:], n % P,
                                    [list(ppr[h][:].ap[0]), [P, 3]])
                        nc.tensor.matmul(
                            outap,
                            hs_sb[:, 256 * i + P * h: 256 * i + P * h + P],
                            spec_s[:], start=True, stop=True)
                if (t + 1) % TPR == 0:
                    r = t // TPR
                    for h in range(2):
                        for c in range(3):
                            nc.scalar.copy(
                                hpT[2 * c + h][:, P * r: P * (r + 1)],
                                ppr[h][:, P * c: P * (c + 1)])

            # ---------------- per-mt blocks ----------------
            def emit_sim_half(mt, mh, sm=None):
                if sm is None:
                    sm = psp_mm.tile([P, 1024], F32, tag="mm",
                                     name=f"sim{mt}")
                for kc in range(6):
                    nc.tensor.matmul(
                        sm[:, 512 * mh: 512 * mh + 512],
                        hpT[kc][:, P * mt: P * mt + P],
                        mpT[kc][:, 512 * mh: 512 * mh + 512],
                        start=(kc == 0), stop=(kc == 5))
                return sm

            def emit_chain(mt, sm):
                negmax = strm.tile([P, 1], F32, tag="ngm")
                nc.vector.tensor_reduce(negmax[:], sm[:], axis=AX.X,
                                        op=OP.max, negate=True)
                pt = strm.tile([P, 1024], F32, tag="P")
                rowsum = strm.tile([P, 1], F32, tag="rs")
                nc.scalar.activation(pt[:], sm[:], ACTF.Exp,
                                     bias=negmax[:, 0:1],
                                     accum_out=rowsum[:, 0:1])
                mx8 = strm.tile([P, 8], F32, tag="mx8")
                nc.vector.max(mx8[:], pt[:])
                nc.vector.max_index(idx_t[mt][:], mx8[:], pt[:])
                nc.vector.reciprocal(recip_t[mt][:], rowsum[:])
                for mc in range(8):
                    tp = psp_tr.tile([P, P], F32, tag="tp")
                    nc.tensor.transpose(tp[:], pt[:, P * mc: P * mc + P],
                                        ident[:])
                    nc.vector.tensor_copy(PT[mc][:], tp[:])
                sf = psp_mm.tile([P, 1024], F32, tag="mm", name=f"soft{mt}")
                for mc in range(8):
                    for (off, wd) in ((0, 512), (512, 256)):
                        nc.tensor.matmul(
                            sf[:, off: off + wd],
                            PT[mc][:],
                            mp[:, D * mc + off: D * mc + off + wd]
                            .bitcast(F32R),
                            start=(mc == 0), stop=(mc == 7))
                soft = strm.tile([P, D], F32, tag="sft")
                nc.scalar.mul(soft[:], sf[:, 0:D], recip_t[mt][:, 0:1])
                nc.scalar.dma_start(osoft_d[P * mt: P * (mt + 1), :],
                                    soft[:])

            def emit_io(mt):
                gmsp = gbuf.tile([P, D], F32, tag="gmsp")
                nc.gpsimd.indirect_dma_start(
                    out=gmsp[:], out_offset=None, in_=msp_d[:],
                    in_offset=bass.IndirectOffsetOnAxis(
                        ap=idx_t[mt][:, 0:1], axis=0))
                gmsd = gbuf.tile([P, D], BF16, tag="gmsd")
                nc.gpsimd.indirect_dma_start(
                    out=gmsd[:], out_offset=None, in_=msdt[:],
                    in_offset=bass.IndirectOffsetOnAxis(
                        ap=idx_t[mt][:, 0:1], axis=0))
                ggr = gbuf.tile([P, D], BF16, tag="ggr")
                nc.gpsimd.indirect_dma_start(
                    out=ggr[:], out_offset=None, in_=gradt[:],
                    in_offset=bass.IndirectOffsetOnAxis(
                        ap=idx_t[mt][:, 0:1], axis=0))
                dif = gbuf.tile([P, D], F32, tag="dif")
                nc.vector.tensor_tensor(dif[:], gmsp[:], gmsd[:],
                                        op=OP.subtract)
                nc.sync.dma_start(oms_d[P * mt: P * (mt + 1), :], gmsp[:])
                nc.gpsimd.dma_start(ogr_d[P * mt: P * (mt + 1), :], ggr[:])
                nc.sync.dma_start(odf_d[P * mt: P * (mt + 1), :], dif[:])

            # ---------------- emission schedule ----------------
            emit_conv()
            for t in range(0, 13):
                emit_proj(t)
            emit_mpT()
            emit_proj(13)
            sm0 = emit_sim_half(0, 0)
            emit_proj(14)
            emit_sim_half(0, 1, sm0)
            emit_proj(15)
            emit_chain(0, sm0)
            emit_proj(16)
            emit_grad_tables()
            sm1 = emit_sim_half(1, 0)
            emit_proj(17)
            emit_sim_half(1, 1, sm1)
            emit_proj(18)
            emit_chain(1, sm1)
            for t in range(19, 24):
                emit_proj(t)
            sm2 = emit_sim_half(2, 0)
            emit_proj(24)
            emit_sim_half(2, 1, sm2)
            emit_proj(25)
            emit_chain(2, sm2)
            for t in range(26, NTILES):
                emit_proj(t)
            with tc.tile_wait_until(0.205):
                emit_tables()
                emit_io(0)
            with tc.tile_wait_until(0.215):
                emit_io(1)
            with tc.tile_wait_until(0.225):
                emit_io(2)
            sm3 = emit_sim_half(3, 0)
            emit_sim_half(3, 1, sm3)
            emit_chain(3, sm3)
            emit_io(3)

    nc.compile()
    return nc


_NC = None


def _get_nc():
    global _NC
    if _NC is None:
        _NC = _build()
    return _NC


def _host_inputs(hs, ms, spec, wcv, b, h):
    # hst: [ch, n1, half, q, n2, pj] -> [ch, (n1 n2) x half x (q pj)]
    hsl = hs[b, :, 256 * h: 256 * (h + 1), :]
    hst = hsl.reshape(128, 16, 2, 8, 32, 16).transpose(0, 1, 4, 2, 3, 5)
    hst = np.ascontiguousarray(hst.reshape(128, NLOC * 256))

    # msh halo patches for the FULL sample
    pad = np.zeros((3, 514, 514), np.float32)
    pad[:, 1:513, 1:513] = ms[b]
    s0, s1, s2 = pad.strides
    view = np.lib.stride_tricks.as_strided(
        pad, shape=(3, 32, 32, 18, 18),
        strides=(s0, 16 * s1, 16 * s2, s1, s2))
    msh = view.transpose(1, 2, 0, 3, 4).reshape(1024, 3 * 324)
    msh = np.ascontiguousarray(
        msh.reshape(8, 128, 972).transpose(1, 0, 2).reshape(128, 8 * 972))

    msp = np.ascontiguousarray(
        ms[b].reshape(3, 32, 16, 32, 16).transpose(1, 3, 0, 2, 4)
        .reshape(NFULL, D))
    return {"hst": hst, "msh": msh, "msp": msp, "spec": spec, "wcv": wcv}


def _unpatch(x):
    # [512, 768] -> [3, 256, 512]
    return (x.reshape(16, 32, 3, 16, 16).transpose(2, 0, 3, 1, 4)
            .reshape(3, 256, 512))


def kernel(hs, ms, spectral_matrix, kernel_weight):
    hs = np.asarray(hs, dtype=np.float32)
    ms = np.asarray(ms, dtype=np.float32)
    spec = np.ascontiguousarray(np.asarray(spectral_matrix, np.float32))
    kw = np.asarray(kernel_weight, np.float32)
    wcv = np.ascontiguousarray(
        np.broadcast_to(kw.reshape(1, 27), (P, 27))).astype(np.float32)

    nc = _get_nc()
    in_maps = [_host_inputs(hs, ms, spec, wcv, core // 2, core % 2)
               for core in range(8)]
    res = run_bass_kernel_spmd(nc, in_maps, list(range(8)))
    out = np.empty((4, 12, 512, 512), np.float32)
    for core in range(8):
        b, h = core // 2, core % 2
        r = res.results[core]
        out[b, 0:3, 256 * h: 256 * (h + 1), :] = _unpatch(r["osoft"])
        out[b, 3:6, 256 * h: 256 * (h + 1), :] = _unpatch(r["oms"])
        out[b, 6:9, 256 * h: 256 * (h + 1), :] = _unpatch(r["ogr"])
        out[b, 9:12, 256 * h: 256 * (h + 1), :] = _unpatch(r["odf"])
    return out


# revision 14
# speedup vs baseline: 1.0551x; 1.0551x over previous
"""Trainium2 Bass kernel for the patch-retrieval module (nn_DMB_46737834115118).

Sharding: 8 cores = 4 batch samples x 2 image-row halves (hp/sim/argmax for
the local 512 patches; conv/grad/tables over the full sample).

All device work happens in PATCH layout; the host does the (pure
permutation) image<->patch transforms:
  - hst:  hs rearranged so each (n, half) gives a [128ch x 128pix] stationary
          block; the projection matmul (moving = spectral matrix, 3 cols)
          emits hs_s directly TRANSPOSED into hpT chunks (full fp32).
  - msh:  18x18 halo patches of ms; conv + sobel gradient run on DVE/ACT/Pool
          as per-partition shift-FMAs (no image-layout shuffles).
  - msp:  patchified ms (gather table).
Outputs are written in patch layout ([512, 768] sections) and the host
unpatchifies. The sim/argmax path is exact fp32 (argmax ties are fatal);
the soft matmul runs in fp32r and the gather tables (output-only data) in
bf16 — both far inside the 2e-2 tolerance.

DMA queues: hs stream exclusively on sync/SP; tables+outputs on scalar/ACT;
gathers on gpsimd/Pool - so nothing head-of-line blocks the hs stream.
"""
import numpy as np

import concourse.bass as bass
import concourse.tile as tile
from concourse import bacc, mybir
from concourse.bass_utils import run_bass_kernel_spmd
from concourse.masks import make_identity

F32 = mybir.dt.float32
F32R = mybir.dt.float32r
BF16 = mybir.dt.bfloat16
U32 = mybir.dt.uint32
AX = mybir.AxisListType
OP = mybir.AluOpType
ACTF = mybir.ActivationFunctionType

P = 128
NLOC = 512       # local patches per core
NFULL = 1024     # full-sample patches
D = 768          # 3*16*16
NT = 16          # n's per hs tile
NTILES = NLOC // NT          # 32
TPR = NTILES // 4            # tiles per round (8)


def _ap(base, off, dims):
    return bass.AP(base.tensor, base.offset + off, [list(d) for d in dims])


def _build():
    nc = bacc.Bacc("TRN2", target_bir_lowering=False, debug=False,
                   num_devices=8)

    hst_d = nc.dram_tensor("hst", [P, NLOC * 256], F32,
                           kind="ExternalInput").ap()
    msh_d = nc.dram_tensor("msh", [P, 8 * 972], F32,
                           kind="ExternalInput").ap()
    msp_d = nc.dram_tensor("msp", [NFULL, D], F32, kind="ExternalInput").ap()
    spec_d = nc.dram_tensor("spec", [P, 3], F32, kind="ExternalInput").ap()
    wcv_d = nc.dram_tensor("wcv", [P, 27], F32, kind="ExternalInput").ap()

    gradt = nc.dram_tensor("gradt", [NFULL, D], BF16, kind="Internal").ap()
    msdt = nc.dram_tensor("msdt", [NFULL, D], BF16, kind="Internal").ap()

    osoft_d = nc.dram_tensor("osoft", [NLOC, D], F32,
                             kind="ExternalOutput").ap()
    oms_d = nc.dram_tensor("oms", [NLOC, D], F32, kind="ExternalOutput").ap()
    ogr_d = nc.dram_tensor("ogr", [NLOC, D], F32, kind="ExternalOutput").ap()
    odf_d = nc.dram_tensor("odf", [NLOC, D], F32, kind="ExternalOutput").ap()

    with tile.TileContext(nc) as tc:
        with tc.tile_pool(name="const", bufs=1) as constp, \
             tc.tile_pool(name="persist", bufs=1) as pers, \
             tc.tile_pool(name="hstream", bufs=3) as hstr, \
             tc.tile_pool(name="stream", bufs=2) as strm, \
             tc.tile_pool(name="gbuf", bufs=2) as gbuf, \
             tc.tile_pool(name="pproj", bufs=1, space="PSUM") as psp_pr, \
             tc.tile_pool(name="pmain", bufs=2, space="PSUM") as psp_mm, \
             tc.tile_pool(name="ptr", bufs=2, space="PSUM") as psp_tr:

            spec_s = constp.tile([P, 3], F32, tag="spec")
            wcv_s = constp.tile([P, 27], F32, tag="wcv")
            ident = constp.tile([P, P], F32, tag="ident")
            eps_s = constp.tile([P, 1], F32, tag="eps")

            msh = pers.tile([P, 8 * 972], F32, tag="msh")
            mp = pers.tile([P, 8 * D], F32, tag="mp")          # [p,(mc,d)]
            hpT = [pers.tile([P, NLOC], F32, tag=f"hpT{k}", name=f"hpT{k}")
                   for k in range(6)]
            mpT = [pers.tile([P, NFULL], F32, tag=f"mpT{k}", name=f"mpT{k}")
                   for k in range(6)]
            PT = [pers.tile([P, P], F32R, tag=f"PT{k}", name=f"PT{k}")
                  for k in range(8)]
            gv = pers.tile([P, 2048], F32, tag="gv")
            gh = pers.tile([P, 2048], F32, tag="gh")
            gvb = pers.tile([P, 3 * 2048], BF16, tag="gvb")
            gate = pers.tile([P, 1], F32, tag="gate")
            recip_t = [pers.tile([P, 1], F32, tag=f"rc{k}", name=f"rc{k}")
                       for k in range(4)]
            idx_t = [pers.tile([P, 8], U32, tag=f"ix{k}", name=f"ix{k}")
                     for k in range(4)]
            ppr = [psp_pr.tile([P, 384], F32, tag=f"ppr{h}", name=f"ppr{h}")
                   for h in range(2)]

            # ---- initial DMAs: first hs tiles before everything else ----
            def load_hs(t):
                hs_sb = hstr.tile([P, NT * 256], F32, tag="hs")
                nc.sync.dma_start(hs_sb[:],
                                  hst_d[:, NT * 256 * t: NT * 256 * (t + 1)])
                return hs_sb

            tiles = {0: load_hs(0)}
            nc.scalar.dma_start(spec_s[:], spec_d[:])
            nc.scalar.dma_start(wcv_s[:], wcv_d[:])
            nc.scalar.dma_start(msh[:], msh_d[:])
            nc.vector.memset(eps_s[:], 1e-6)
            make_identity(nc, ident[:])

            # ------------- conv (patch space; <=3D APs per instr) --------
            def halo_in(c, ck, di, dj):
                return _ap(msh[:], 972 * ck + 324 * c + 18 * di + dj,
                           [list(msh[:].ap[0]), [18, 16], [1, 16]])

            def emit_conv():
                # c=0,1 on DVE; c=2 on GPSIMD (keeps conv latency low so the
                # mpT transposes are ready before sim0)
                for c in range(3):
                    eng = nc.gpsimd if c == 2 else nc.vector
                    for ck in range(8):
                        out = mp[:, D * ck + 256 * c: D * ck + 256 * c + 256]
                        first = True
                        for di in range(3):
                            for dj in range(3):
                                wap = wcv_s[:, 9 * c + 3 * di + dj:
                                            9 * c + 3 * di + dj + 1]
                                src = halo_in(c, ck, di, dj)
                                if first:
                                    eng.tensor_scalar_mul(out, src, wap)
                                    first = False
                                else:
                                    eng.scalar_tensor_tensor(
                                        out, src, wap, out,
                                        op0=OP.mult, op1=OP.add)

            # grad + tables (emitted late so ACT stays free early)
            def emit_grad_tables():
                for c in range(3):
                    for ck in range(8):
                        nc.vector.tensor_tensor(
                            gv[:, 256 * ck: 256 * ck + 256],
                            halo_in(c, ck, 2, 1), halo_in(c, ck, 0, 1),
                            op=OP.subtract)
                        nc.vector.tensor_tensor(
                            gh[:, 256 * ck: 256 * ck + 256],
                            halo_in(c, ck, 1, 2), halo_in(c, ck, 1, 0),
                            op=OP.subtract)
                    nc.scalar.square(gv[:], gv[:])
                    nc.vector.tensor_tensor(gh[:], gh[:], gh[:], op=OP.mult)
                    nc.vector.tensor_tensor(gv[:], gv[:], gh[:], op=OP.add)
                    nc.scalar.activation(gvb[:, 2048 * c: 2048 * (c + 1)],
                                         gv[:], ACTF.Sqrt,
                                         bias=eps_s[:, 0:1])

            def emit_tables():
                # gate Pool's deferred DMAs behind the round-3 evacuation
                # (real dep on the last hs tile) so they fill the tail window
                nc.gpsimd.tensor_copy(gate[:], hpT[5][:, 511:512])
                for c in range(3):
                    dst = _ap(gradt[:], 256 * c,
                              [[D, P], [P * D, 8], [1, 256]])
                    src = _ap(gvb[:], 2048 * c,
                              [list(gvb[:].ap[0]), [256, 8], [1, 256]])
                    nc.scalar.dma_start(dst, src)
                for g in range(4):          # msdt in 4 pieces (2 chunks
                    # each), f32 -> bf16 cast during SWDGE DMA
                    dstm = _ap(msdt[:], 2 * P * D * g,
                               [[D, P], [P * D, 2], [1, D]])
                    srcm = _ap(mp[:], 2 * D * g,
                               [list(mp[:].ap[0]), [D, 2], [1, D]])
                    nc.gpsimd.dma_start(dstm, srcm)

            def emit_mpT():
                for kc in range(6):
                    for j in range(8):
                        tp = psp_tr.tile([P, P], F32, tag="tp")
                        nc.tensor.transpose(
                            tp[:], mp[:, D * j + P * kc: D * j + P * kc + P],
                            ident[:])
                        nc.vector.tensor_copy(
                            mpT[kc][:, P * j: P * j + P], tp[:])
                # all exact consumers of mp are emitted; round in place for
                # the (tolerance-loose) fp32r soft matmuls
                nc.vector.tensor_copy(mp[:].bitcast(F32R), mp[:])

            # ---------------- hs projection for one tile ----------------
            def emit_proj(t):
                hs_sb = tiles.pop(t)
                if t + 1 < NTILES:
                    tiles[t + 1] = load_hs(t + 1)
                for i in range(NT):
                    n = NT * t + i
                    for h in range(2):
                        outap = _ap(ppr[h][:], n % P,
                                    [list(ppr[h][:].ap[0]), [P, 3]])
                        nc.tensor.matmul(
                            outap,
                            hs_sb[:, 256 * i + P * h: 256 * i + P * h + P],
                            spec_s[:], start=True, stop=True)
                if (t + 1) % TPR == 0:
                    r = t // TPR
                    for h in range(2):
                        for c in range(3):
                            nc.scalar.copy(
                                hpT[2 * c + h][:, P * r: P * (r + 1)],
                                ppr[h][:, P * c: P * (c + 1)])

            # ---------------- per-mt blocks ----------------
            def emit_sim_half(mt, mh, sm=None):
                if sm is None:
                    sm = psp_mm.tile([P, 1024], F32, tag="mm",
                                     name=f"sim{mt}")
                for kc in range(6):
                    nc.tensor.matmul(
                        sm[:, 512 * mh: 512 * mh + 512],
                        hpT[kc][:, P * mt: P * mt + P],
                        mpT[kc][:, 512 * mh: 512 * mh + 512],
                        start=(kc == 0), stop=(kc == 5))
                return sm

            def emit_chain(mt, sm):
                negmax = strm.tile([P, 1], F32, tag="ngm")
                nc.vector.tensor_reduce(negmax[:], sm[:], axis=AX.X,
                                        op=OP.max, negate=True)
                pt = strm.tile([P, 1024], F32, tag="P")
                rowsum = strm.tile([P, 1], F32, tag="rs")
                nc.scalar.activation(pt[:], sm[:], ACTF.Exp,
                                     bias=negmax[:, 0:1],
                                     accum_out=rowsum[:, 0:1])
                mx8 = strm.tile([P, 8], F32, tag="mx8")
                nc.vector.max(mx8[:], pt[:])
                nc.vector.max_index(idx_t[mt][:], mx8[:], pt[:])
                nc.vector.reciprocal(recip_t[mt][:], rowsum[:])
                for mc in range(8):
                    tp = psp_tr.tile([P, P], F32, tag="tp")
                    nc.tensor.transpose(tp[:], pt[:, P * mc: P * mc + P],
                                        ident[:])
                    nc.vector.tensor_copy(PT[mc][:], tp[:])
                sf = psp_mm.tile([P, 1024], F32, tag="mm", name=f"soft{mt}")
                for mc in range(8):
                    for (off, wd) in ((0, 512), (512, 256)):
                        nc.tensor.matmul(
                            sf[:, off: off + wd],
                            PT[mc][:],
                            mp[:, D * mc + off: D * mc + off + wd]
                            .bitcast(F32R),
                            start=(mc == 0), stop=(mc == 7))
                soft = strm.tile([P, D], F32, tag="sft")
                nc.scalar.mul(soft[:], sf[:, 0:D], recip_t[mt][:, 0:1])
                nc.scalar.dma_start(osoft_d[P * mt: P * (mt + 1), :],
                                    soft[:])

            def emit_io(mt):
                gmsp = gbuf.tile([P, D], F32, tag="gmsp")
                nc.gpsimd.indirect_dma_start(
                    out=gmsp[:], out_offset=None, in_=msp_d[:],
                    in_offset=bass.IndirectOffsetOnAxis(
                        ap=idx_t[mt][:, 0:1], axis=0))
                gmsd = gbuf.tile([P, D], BF16, tag="gmsd")
                nc.gpsimd.indirect_dma_start(
                    out=gmsd[:], out_offset=None, in_=msdt[:],
                    in_offset=bass.IndirectOffsetOnAxis(
                        ap=idx_t[mt][:, 0:1], axis=0))
                ggr = gbuf.tile([P, D], BF16, tag="ggr")
                nc.gpsimd.indirect_dma_start(
                    out=ggr[:], out_offset=None, in_=gradt[:],
                    in_offset=bass.IndirectOffsetOnAxis(
                        ap=idx_t[mt][:, 0:1], axis=0))
                dif = gbuf.tile([P, D], F32, tag="dif")
                nc.vector.tensor_tensor(dif[:], gmsp[:], gmsd[:],
                                        op=OP.subtract)
                nc.sync.dma_start(oms_d[P * mt: P * (mt + 1), :], gmsp[:])
                nc.gpsimd.dma_start(ogr_d[P * mt: P * (mt + 1), :], ggr[:])
                nc.sync.dma_start(odf_d[P * mt: P * (mt + 1), :], dif[:])

            # ---------------- emission schedule ----------------
            emit_conv()
            for t in range(0, 13):
                emit_proj(t)
            emit_mpT()
            emit_proj(13)
            sm0 = emit_sim_half(0, 0)
            emit_proj(14)
            emit_sim_half(0, 1, sm0)
            emit_proj(15)
            emit_chain(0, sm0)
            emit_proj(16)
            emit_grad_tables()
            sm1 = emit_sim_half(1, 0)
            emit_proj(17)
            emit_sim_half(1, 1, sm1)
            emit_proj(18)
            emit_chain(1, sm1)
            for t in range(19, 24):
                emit_proj(t)
            sm2 = emit_sim_half(2, 0)
            emit_proj(24)
            emit_sim_half(2, 1, sm2)
            emit_proj(25)
            emit_chain(2, sm2)
            for t in range(26, NTILES):
                emit_proj(t)
            with tc.tile_wait_until(0.205):
                emit_tables()
                emit_io(0)
            with tc.tile_wait_until(0.215):
                emit_io(1)
            with tc.tile_wait_until(0.225):
                emit_io(2)
            sm3 = emit_sim_half(3, 0)
            emit_sim_half(3, 1, sm3)
            emit_chain(3, sm3)
            emit_io(3)

    nc.compile()
    return nc


_NC = None


def _get_nc():
    global _NC
    if _NC is None:
        _NC = _build()
    return _NC


def _host_inputs(hs, ms, spec, wcv, b, h):
    # hst: [ch, n1, half, q, n2, pj] -> [ch, (n1 n2) x half x (q pj)]
    hsl = hs[b, :, 256 * h: 256 * (h + 1), :]
    hst = hsl.reshape(128, 16, 2, 8, 32, 16).transpose(0, 1, 4, 2, 3, 5)
    hst = np.ascontiguousarray(hst.reshape(128, NLOC * 256))

    # msh halo patches for the FULL sample
    pad = np.zeros((3, 514, 514), np.float32)
    pad[:, 1:513, 1:513] = ms[b]
    s0, s1, s2 = pad.strides
    view = np.lib.stride_tricks.as_strided(
        pad, shape=(3, 32, 32, 18, 18),
        strides=(s0, 16 * s1, 16 * s2, s1, s2))
    msh = view.transpose(1, 2, 0, 3, 4).reshape(1024, 3 * 324)
    msh = np.ascontiguousarray(
        msh.reshape(8, 128, 972).transpose(1, 0, 2).reshape(128, 8 * 972))

    msp = np.ascontiguousarray(
        ms[b].reshape(3, 32, 16, 32, 16).transpose(1, 3, 0, 2, 4)
        .reshape(NFULL, D))
    return {"hst": hst, "msh": msh, "msp": msp, "spec": spec, "wcv": wcv}


def _unpatch(x):
    # [512, 768] -> [3, 256, 512]
    return (x.reshape(16, 32, 3, 16, 16).transpose(2, 0, 3, 1, 4)
            .reshape(3, 256, 512))


def kernel(hs, ms, spectral_matrix, kernel_weight):
    hs = np.asarray(hs, dtype=np.float32)
    ms = np.asarray(ms, dtype=np.float32)
    spec = np.ascontiguousarray(np.asarray(spectral_matrix, np.float32))
    kw = np.asarray(kernel_weight, np.float32)
    wcv = np.ascontiguousarray(
        np.broadcast_to(kw.reshape(1, 27), (P, 27))).astype(np.float32)

    nc = _get_nc()
    in_maps = [_host_inputs(hs, ms, spec, wcv, core // 2, core % 2)
               for core in range(8)]
    res = run_bass_kernel_spmd(nc, in_maps, list(range(8)))
    out = np.empty((4, 12, 512, 512), np.float32)
    for core in range(8):
        b, h = core // 2, core % 2
        r = res.results[core]
        out[b, 0:3, 256 * h: 256 * (h + 1), :] = _unpatch(r["osoft"])
        out[b, 3:6, 256 * h: 256 * (h + 1), :] = _unpatch(r["oms"])
        out[b, 6:9, 256 * h: 256 * (h + 1), :] = _unpatch(r["ogr"])
        out[b, 9:12, 256 * h: 256 * (h + 1), :] = _unpatch(r["odf"])
    return out
